# revision 1
# baseline (speedup 1.0000x reference)
"""Trainium2 Bass kernel for nn_Attention_49185965473844.

Math (per example b):
    q = x @ Wq ; k = x @ Wk ; v = x @ Wv          (x: [S, D], W*: [D, D], D=32)
    A[q,k]   = sum_s q[s,q] k[s,k]  = (Wq^T G Wk)[q,k],   G = x^T x   ([32, 32])
    scores   = softmax(A, axis=q)                 (normalize down columns)
    out[q,s] = sum_k scores[q,k] v[s,k] = (M @ x^T)[q,s], M = scores @ Wv^T

So the whole problem reduces to: one Gram matrix G = x^T x per example (the
only big contraction, streamed over S), a tiny 32x32 chain + softmax, and one
[32,32] @ [32,S] matmul against x^T (PE transposes of the resident x tile).

Sharding: pure data parallel over batch B=64 -> 8 examples per NeuronCore.
"""

import numpy as np

import concourse.bass as bass
import concourse.bacc as bacc
import concourse.tile as tile
from concourse import mybir
from concourse.bass_utils import run_bass_kernel_spmd

N_CORES = 8
B, S, D = 64, 8192, 32
PER_CORE = B // N_CORES  # 8

F32 = mybir.dt.float32
F32R = mybir.dt.float32r

# float32r (TF32-like reduced-precision PE matmul mode; 1 cyc/row at moving
# dim >= 256 vs fp32's 4, single-pass weight load) for the Gram, transpose
# and output matmuls. walrus requires f32r matmul inputs to come from
# producers with f32r output dtype: x/eye are declared f32r at the DRAM
# level (bits are plain fp32), and the M^T block-diagonal picks up f32r in
# its masking multiply. Measured end-to-end relative error ~3e-4.
USE_F32R = True


def build_nc(n_ex=PER_CORE, seq=S):
    """Build the per-core Bass program. Same program runs on all 8 cores.

    s-index decomposition: s = 128*c + p, chunk c = 4*t + j (quad t, partition
    block j), quad t = 4*g + h (store group g).  So
        s = 2048*g + 512*h + 128*j + p,   h in [0,4), j in [0,4), p in [0,128).
    The PE transpose of natural-tile quad t produces partition (j, d), free p;
    the final block-diagonal matmul then yields out rows (j, q) and free
    (h, p) per group — stored with one 3-dim DMA per (g, j), spread over the
    scalar/sync HWDGE rings and the gpsimd SWDGE ring.
    """
    assert seq % 2048 == 0
    n_chunks = seq // 128     # 128-row chunks of x
    n_quads = n_chunks // 4   # [128, 128] column blocks of the natural tile
    n_groups = n_quads // 4   # store groups: 4 quads -> [128, 512] out tiles

    nc = bacc.Bacc("TRN2", target_bir_lowering=False, debug=False)
    # x is declared float32r: bits are plain fp32; the PE's f32r matmuls
    # truncate mantissas internally (TF32-like). This satisfies the
    # birverifier's "rounded producer" rule via the dma's f32r output dtype.
    x_t = nc.declare_dram_parameter("x", [n_ex, seq, D], F32R, isOutput=False)
    eye_t = nc.declare_dram_parameter("eye", [128, 128], F32R, isOutput=False)
    cst_t = nc.declare_dram_parameter("cst", [128, 352], F32, isOutput=False)
    out_t = nc.declare_dram_parameter("out", [n_ex, D, seq], F32, isOutput=True)

    with tile.TileContext(nc) as tc:
        with (
            tc.tile_pool(name="consts", bufs=1) as consts,
            tc.tile_pool(name="nat_pool", bufs=n_ex) as nat_pool,
            tc.tile_pool(name="trhs_pool", bufs=4) as trhs_pool,
            tc.tile_pool(name="osb_pool", bufs=n_ex * n_groups) as osb_pool,
            tc.tile_pool(name="small_pool", bufs=3) as small_pool,
            tc.tile_pool(name="acc_psum", bufs=2, space="PSUM") as acc_psum,
            tc.tile_pool(name="tp_psum", bufs=1, space="PSUM") as tp_psum,
            tc.tile_pool(name="o_psum", bufs=2, space="PSUM") as o_psum,
        ):
            # ---- constants: one DMA so every PE consumer has a single
            # upstream sync (fp32 matmuls only get 1 sync wait in walrus) ----
            cst_sb = consts.tile([128, 352], F32)
            nc.sync.dma_start(out=cst_sb, in_=cst_t[:, :])
            identity = cst_sb[:, 0:128]
            wv4 = cst_sb[:, 128:160]       # np.tile(Wv, (4, 1))
            wq_sb = cst_sb[0:D, 160:192]
            wk_sb = cst_sb[0:D, 192:224]
            blkmask = cst_sb[:, 224:352]   # [p, c] = 1.0 iff p//32 == c//32
            # Wv replicated on 4 partition blocks, PE-transposed so that
            # wvt_rep[k, 32*j + d] = Wv[d, k].
            wvt_ps = acc_psum.tile([D, 128], F32, tag="acc")
            nc.tensor.transpose(wvt_ps, wv4, identity)
            wvt_rep = consts.tile([D, 128], F32)
            nc.scalar.copy(out=wvt_rep, in_=wvt_ps)
            ident_r = consts.tile([128, 128], F32R)
            nc.sync.dma_start(out=ident_r, in_=eye_t[:, :])

            def make_tp(nat2, g):
                """PE-transpose quads t = 4g + (0..3) into one PSUM bank."""
                tp_ps = tp_psum.tile(
                    [128, 512], F32, tag=f"tp{g}", bufs=1, name=f"tp_{g}"
                )
                for i in range(4):
                    t = 4 * g + i
                    nc.tensor.transpose(
                        tp_ps[:, 128 * i:128 * (i + 1)].bitcast(F32R),
                        nat2[:, 128 * t:128 * (t + 1)],
                        ident_r,
                    )
                return tp_ps

            def load_nat(b):
                # x_b as [128, n_chunks * 32]; chunk c col-block holds
                # x[128*c + p, :] on partition p
                nat = nat_pool.tile([128, n_chunks, D], F32R, tag="nat",
                                    name=f"nat_{b}")
                nc.sync.dma_start(
                    out=nat, in_=x_t[b].rearrange("(c p) d -> p c d", p=128)
                )
                return nat

            def emit_o_phase(b, tp_tiles, bd):
                """Output phase (trhs copies, block-diag matmuls, stores)
                for example b; copy engines alternate by group parity."""
                for g in range(n_groups):
                    trhs = trhs_pool.tile([128, 512], F32R if USE_F32R else F32,
                                          tag="trhs")
                    tp_src = tp_tiles[g].bitcast(F32R) if USE_F32R else tp_tiles[g]
                    if g % 2 == 0:
                        nc.scalar.copy(out=trhs, in_=tp_src)
                    else:
                        nc.vector.tensor_copy(out=trhs, in_=tp_src)
                    o_ps = o_psum.tile([128, 512], F32, tag="o")
                    nc.tensor.matmul(o_ps, lhsT=bd, rhs=trhs)
                    o_sb = osb_pool.tile([128, 512], F32, tag="o_sb")
                    if g % 2 == 0:
                        nc.scalar.copy(out=o_sb, in_=o_ps)
                    else:
                        nc.vector.tensor_copy(out=o_sb, in_=o_ps)
                    dst4 = out_t[b].rearrange(
                        "q (gg h j p) -> gg j q h p",
                        gg=n_groups, h=4, j=4, p=128,
                    )[g]
                    steng = nc.scalar if g % 2 == 0 else nc.sync
                    for j in range(4):
                        eng = steng if j < 2 else nc.gpsimd
                        eng.dma_start(
                            out=dst4[j], in_=o_sb[32 * j:32 * (j + 1), :]
                        )

            o_state = {}

            # All example loads are queued upfront (x is SBUF-resident for
            # the whole kernel): the DMA engines always have load packets
            # available, stores interleave at packet granularity, and no
            # load's descriptor generation ever queues behind store waits.
            nats = {b: load_nat(b) for b in range(n_ex)}
            for b in range(n_ex):
                nat = nats.pop(b)
                nat2 = nat.rearrange("p c d -> p (c d)")

                # ---- Gram accumulation: 128x128 of quad cross-products;
                # the 4 diagonal 32x32 blocks sum to G = x^T x ----
                # f32r runs 1 cyc/row only at moving dim >= 256: moving
                # block = quad pair, valid self-product in the left half.
                gram_ps = acc_psum.tile([128, 256], F32, tag="acc")
                for t in range(n_quads):
                    last = t == n_quads - 1
                    width = 128 if last else 256
                    nc.tensor.matmul(
                        gram_ps[:, 0:width],
                        lhsT=nat2[:, 128 * t:128 * (t + 1)],
                        rhs=nat2[:, 128 * t:128 * t + width],
                        start=(t == 0),
                        stop=last,
                        skip_group_check=True,
                    )

                # PE does group-0/1 transposes while ACT folds gram to SBUF.
                tp_tiles = {}
                tp_tiles[0] = make_tp(nat2, 0)
                tp_tiles[1] = make_tp(nat2, 1)

                # ---- fold the 4 diagonal 32x32 blocks of gram into G ----
                gram_sb = small_pool.tile([128, 128], F32, tag="gram_sb")
                nc.scalar.copy(out=gram_sb, in_=gram_ps[:, 0:128])
                g_ps = acc_psum.tile([D, D], F32, tag="acc")
                for j in range(4):
                    nc.tensor.matmul(
                        g_ps,
                        lhsT=identity[:, 32 * j:32 * (j + 1)],
                        rhs=gram_sb[:, 32 * j:32 * (j + 1)],
                        start=(j == 0),
                        stop=(j == 3),
                    )
                g_sb = small_pool.tile([D, D], F32, tag="g_sb")
                nc.scalar.copy(out=g_sb, in_=g_ps)

                # ---- A^T = Wk^T (G Wq);  G symmetric so lhsT=G works ----
                t2_ps = acc_psum.tile([D, D], F32, tag="acc")
                nc.tensor.matmul(t2_ps, lhsT=g_sb, rhs=wq_sb)
                t2_sb = small_pool.tile([D, D], F32, tag="t2_sb")
                nc.scalar.copy(out=t2_sb, in_=t2_ps)
                at_ps = acc_psum.tile([D, D], F32, tag="acc")
                nc.tensor.matmul(at_ps, lhsT=wk_sb, rhs=t2_sb)

                # ---- softmax over q (free dim of A^T), on DVE/ACT while the
                # PE runs the remaining transposes ----
                nmax = small_pool.tile([D, 1], F32, tag="nmax")
                nc.vector.reduce_max(
                    out=nmax, in_=at_ps, axis=mybir.AxisListType.X, negate=True
                )
                e_sb = small_pool.tile([D, D], F32, tag="e_sb")
                rsum = small_pool.tile([D, 1], F32, tag="rsum")
                # exp and its row-sum fused in one ACT instruction (accum_out)
                nc.scalar.activation(
                    out=e_sb, in_=at_ps,
                    func=mybir.ActivationFunctionType.Exp,
                    bias=nmax, scale=1.0,
                    accum_out=rsum,
                )
                rinv = small_pool.tile([D, 1], F32, tag="rinv")
                nc.vector.reciprocal(out=rinv, in_=rsum)
                sc_sb = small_pool.tile([D, D], F32, tag="sc_sb")
                nc.vector.tensor_scalar_mul(out=sc_sb, in0=e_sb, scalar1=rinv)

                for g in range(2, n_groups):
                    tp_tiles[g] = make_tp(nat2, g)

                # ---- M^T replicated on 4 partition blocks ----
                m4_ps = acc_psum.tile([128, D], F32, tag="acc")
                nc.tensor.matmul(m4_ps, lhsT=wvt_rep, rhs=sc_sb)
                m4_sb = small_pool.tile([128, D], F32, tag="m4_sb")
                nc.scalar.copy(out=m4_sb, in_=m4_ps)
                # Block-diagonal lhsT for the output matmuls: one full-width
                # matmul per group instead of four 32x32 sub-tile matmuls
                # (walrus rejects f32r + tile_position). The mask multiply
                # also performs the f32r rounding.
                bd = small_pool.tile([128, 128], F32R if USE_F32R else F32,
                                     tag="bd")
                m4_bcast = bass.AP(
                    tensor=m4_sb.tensor,
                    offset=m4_sb.offset,
                    ap=[list(m4_sb.ap[0]), [0, 4], list(m4_sb.ap[1])],
                )
                nc.gpsimd.tensor_mul(
                    out=bd.rearrange("p (r q) -> p r q", r=4),
                    in0=m4_bcast,
                    in1=blkmask.rearrange("p (r q) -> p r q", r=4),
                )

                emit_o_phase(b, tp_tiles, bd)

    nc.compile()
    return nc


_CACHED_NC = None


def _get_nc():
    global _CACHED_NC
    if _CACHED_NC is None:
        _CACHED_NC = build_nc()
    return _CACHED_NC


def make_cst(wq, wk, wv):
    """[128, 352]: identity | tile(Wv,(4,1)) | Wq | Wk | 32x32 block mask."""
    cst = np.zeros((128, 352), dtype=np.float32)
    cst[:, 0:128] = np.eye(128, dtype=np.float32)
    cst[:, 128:160] = np.tile(wv, (4, 1))
    cst[0:D, 160:192] = wq
    cst[0:D, 192:224] = wk
    blk = np.arange(128) // 32
    cst[:, 224:352] = (blk[:, None] == blk[None, :]).astype(np.float32)
    return cst


def kernel(x, Wq, Wk, Wv):
    x = np.ascontiguousarray(np.asarray(x, dtype=np.float32))
    wq = np.asarray(Wq, dtype=np.float32).reshape(D, D)
    wk = np.asarray(Wk, dtype=np.float32).reshape(D, D)
    wv = np.asarray(Wv, dtype=np.float32).reshape(D, D)
    assert x.shape == (B, S, D)
    cst = make_cst(wq, wk, wv)

    nc = _get_nc()
    eye = np.eye(128, dtype=np.float32)
    in_maps = [
        {
            "x": x[c * PER_CORE:(c + 1) * PER_CORE],
            "cst": cst,
            "eye": eye,
        }
        for c in range(N_CORES)
    ]
    res = run_bass_kernel_spmd(nc, in_maps, list(range(N_CORES)))
    out = np.concatenate([res.results[c]["out"] for c in range(N_CORES)], axis=0)
    return out



# revision 7
# speedup vs baseline: 1.0445x; 1.0445x over previous
"""Trainium2 Bass kernel for nn_Attention_49185965473844.

Math (per example b):
    q = x @ Wq ; k = x @ Wk ; v = x @ Wv          (x: [S, D], W*: [D, D], D=32)
    A[q,k]   = sum_s q[s,q] k[s,k]  = (Wq^T G Wk)[q,k],   G = x^T x   ([32, 32])
    scores   = softmax(A, axis=q)                 (normalize down columns)
    out[q,s] = sum_k scores[q,k] v[s,k] = (M @ x^T)[q,s], M = scores @ Wv^T

So the whole problem reduces to: one Gram matrix G = x^T x per example (the
only big contraction, streamed over S), a tiny 32x32 chain + softmax, and one
[32,32] @ [32,S] matmul against x^T (PE transposes of the resident x tile).

The kernel is HBM/DMA-packet-bound, so the SBUF layout of x is chosen to give
large contiguous DMA descriptors on BOTH the load and the store:

    s = 2048*c + 16*p + l,   c in [0,4), p in [0,128) (partition), l in [0,16)

  * load:  nat[p, (c,l,d)] = x[s,d]  -> per partition 4 runs of 16 rows
    = 2 KB contiguous each (vs 128 B in a plain "(c p) d" layout).
  * PE transpose of the [128, (c d)] column block at fixed l gives
    T_l[(c,d), p] = x^T with partition group c = the TOP 2 bits of s.
  * block-diag matmul (bd columns ordered (q, g)) -> o[(q,c), p].
  * the mandatory PSUM->SBUF copy scatters columns p -> 16*p + l, so the
    assembled O_sb[(q,c), f] = out[q, 2048*c + f] stores as ONE fully
    contiguous 1 MB DMA per example.

Sharding: pure data parallel over batch B=64 -> 8 examples per NeuronCore.
"""

import numpy as np

import concourse.bass as bass
import concourse.bacc as bacc
import concourse.tile as tile
from concourse import mybir
from concourse.bass_utils import run_bass_kernel_spmd

N_CORES = 8
B, S, D = 64, 8192, 32
PER_CORE = B // N_CORES  # 8

F32 = mybir.dt.float32
F32R = mybir.dt.float32r

# float32r (TF32-like reduced-precision PE matmul mode; 1 cyc/row at moving
# dim >= 256 vs fp32's 4, 1.5 cyc/row transposes) for the Gram, transpose
# and output matmuls. walrus requires f32r matmul inputs to come from
# producers with f32r output dtype: x/eye are declared f32r at the DRAM
# level (bits are plain fp32), and the bd block-diagonal picks up f32r in
# its masking multiply. Measured end-to-end relative error ~3e-4.

N_C = 4    # s bits 11..12: partition group of the transposed tiles
N_L = 16   # s bits 0..3:  within-partition interleave (load run = 16 rows)
N_P = 128  # s bits 4..10: SBUF partition of the natural tile


def build_nc(n_ex=PER_CORE, seq=S):
    """Build the per-core Bass program. Same program runs on all 8 cores."""
    assert seq == N_C * N_P * N_L
    nc = bacc.Bacc("TRN2", target_bir_lowering=False, debug=False)
    # x is declared float32r: bits are plain fp32; the PE's f32r matmuls
    # truncate mantissas internally (TF32-like). This satisfies the
    # birverifier's "rounded producer" rule via the dma's f32r output dtype.
    x_t = nc.declare_dram_parameter("x", [n_ex, seq, D], F32R, isOutput=False)
    eye_t = nc.declare_dram_parameter("eye", [128, 128], F32R, isOutput=False)
    cst_t = nc.declare_dram_parameter("cst", [128, 352], F32, isOutput=False)
    out_t = nc.declare_dram_parameter("out", [n_ex, D, seq], F32, isOutput=True)

    with tile.TileContext(nc) as tc:
        with (
            tc.tile_pool(name="consts", bufs=1) as consts,
            tc.tile_pool(name="nat_pool", bufs=n_ex) as nat_pool,
            tc.tile_pool(name="natp_pool", bufs=3) as natp_pool,
            tc.tile_pool(name="trhs_pool", bufs=4) as trhs_pool,
            tc.tile_pool(name="osb_pool", bufs=n_ex) as osb_pool,
            tc.tile_pool(name="small_pool", bufs=3) as small_pool,
            tc.tile_pool(name="acc_psum", bufs=2, space="PSUM") as acc_psum,
            tc.tile_pool(name="tp_psum", bufs=2, space="PSUM") as tp_psum,
            tc.tile_pool(name="o_psum", bufs=2, space="PSUM") as o_psum,
        ):
            # ---- constants: one DMA so every PE consumer has a single
            # upstream sync (fp32 matmuls only get 1 sync wait in walrus) ----
            cst_sb = consts.tile([128, 352], F32)
            nc.sync.dma_start(out=cst_sb, in_=cst_t[:, :])
            identity = cst_sb[:, 0:128]
            wv4 = cst_sb[:, 128:160]       # np.tile(Wv, (4, 1))
            wq_sb = cst_sb[0:D, 160:192]
            wk_sb = cst_sb[0:D, 192:224]
            # qgmask[p, 4*q + g] = 1.0 iff p//32 == g
            qgmask = cst_sb[:, 224:352]
            # Wv replicated on 4 partition blocks, PE-transposed so that
            # wvt_rep[k, 32*j + d] = Wv[d, k].
            wvt_ps = acc_psum.tile([D, 128], F32, tag="acc")
            nc.tensor.transpose(wvt_ps, wv4, identity)
            wvt_rep = consts.tile([D, 128], F32)
            nc.scalar.copy(out=wvt_rep, in_=wvt_ps)
            ident_r = consts.tile([128, 128], F32R)
            nc.sync.dma_start(out=ident_r, in_=eye_t[:, :])

            def load_nat(b):
                # nat[p, c, l, d] = x[b, 2048c + 16p + l, d]; per partition
                # the (l, d) block is 16 rows = 2 KB contiguous in DRAM.
                nat = nat_pool.tile([128, N_C, N_L, D], F32R, tag="nat",
                                    name=f"nat_{b}")
                nc.sync.dma_start(
                    out=nat,
                    in_=x_t[b].rearrange("(c p l) d -> p c l d",
                                         c=N_C, p=N_P, l=N_L),
                )
                return nat

            def make_tp(natP2, t):
                """PE-transpose the [128, (c d)] blocks at l = 4t+i into one
                PSUM bank; T[(c,d), p] = x[2048c + 16p + l, d]."""
                tp_ps = tp_psum.tile([128, 512], F32, tag="tp",
                                     name=f"tp_{t}")
                for i in range(4):
                    l0 = 4 * t + i
                    nc.tensor.transpose(
                        tp_ps[:, 128 * i:128 * (i + 1)].bitcast(F32R),
                        natP2[:, 128 * l0:128 * (l0 + 1)],
                        ident_r,
                    )
                return tp_ps

            o_sb_tiles = {}

            # All example loads are queued upfront (x is SBUF-resident for
            # the whole kernel): the DMA engines always have load packets
            # available and stores (on the gpsimd queue) interleave at
            # packet granularity.
            nats = {b: load_nat(b) for b in range(n_ex)}
            for b in range(n_ex):
                nat = nats.pop(b)
                # Reorder (c,l,d) -> (l,c,d) on the Pool engine so each
                # transpose block is 128 contiguous columns (the PE moving
                # operand must be a single-stride AP). The DMA itself cannot
                # do this permutation without breaking its 2 KB descriptors.
                natP = natp_pool.tile([128, N_L, N_C, D], F32R, tag="natp")
                nc.gpsimd.tensor_copy(
                    out=natP, in_=nat.rearrange("p c l d -> p l c d")
                )
                natP2 = natP.rearrange("p l c d -> p (l c d)")

                # ---- Gram accumulation: 128x128 of row-group cross
                # products; the 4 diagonal 32x32 blocks sum to G = x^T x ----
                # f32r runs 1 cyc/row only at moving dim >= 256: moving
                # block = column-block pair, valid self-product in the left
                # half.
                n_quads = (N_C * N_L * D) // 128  # 16
                gram_ps = acc_psum.tile([128, 256], F32, tag="acc")
                for t in range(n_quads):
                    last = t == n_quads - 1
                    width = 128 if last else 256
                    nc.tensor.matmul(
                        gram_ps[:, 0:width],
                        lhsT=natP2[:, 128 * t:128 * (t + 1)],
                        rhs=natP2[:, 128 * t:128 * t + width],
                        start=(t == 0),
                        stop=last,
                        skip_group_check=True,
                    )

                # PE does batch-0/1 transposes while ACT folds gram to SBUF.
                tp_tiles = {}
                tp_tiles[0] = make_tp(natP2, 0)

                # ---- fold the 4 diagonal 32x32 blocks of gram into G ----
                gram_sb = small_pool.tile([128, 128], F32, tag="gram_sb")
                nc.scalar.copy(out=gram_sb, in_=gram_ps[:, 0:128])
                g_ps = acc_psum.tile([D, D], F32, tag="acc")
                for j in range(4):
                    nc.tensor.matmul(
                        g_ps,
                        lhsT=identity[:, 32 * j:32 * (j + 1)],
                        rhs=gram_sb[:, 32 * j:32 * (j + 1)],
                        start=(j == 0),
                        stop=(j == 3),
                    )
                g_sb = small_pool.tile([D, D], F32, tag="g_sb")
                nc.scalar.copy(out=g_sb, in_=g_ps)

                tp_tiles[1] = make_tp(natP2, 1)

                # ---- A^T = Wk^T (G Wq);  G symmetric so lhsT=G works ----
                t2_ps = acc_psum.tile([D, D], F32, tag="acc")
                nc.tensor.matmul(t2_ps, lhsT=g_sb, rhs=wq_sb)
                t2_sb = small_pool.tile([D, D], F32, tag="t2_sb")
                nc.scalar.copy(out=t2_sb, in_=t2_ps)
                at_ps = acc_psum.tile([D, D], F32, tag="acc")
                nc.tensor.matmul(at_ps, lhsT=wk_sb, rhs=t2_sb)

                # ---- softmax over q (free dim of A^T), on DVE/ACT while the
                # PE runs the remaining transposes ----
                nmax = small_pool.tile([D, 1], F32, tag="nmax")
                nc.vector.reduce_max(
                    out=nmax, in_=at_ps, axis=mybir.AxisListType.X, negate=True
                )
                e_sb = small_pool.tile([D, D], F32, tag="e_sb")
                rsum = small_pool.tile([D, 1], F32, tag="rsum")
                # exp and its row-sum fused in one ACT instruction (accum_out)
                nc.scalar.activation(
                    out=e_sb, in_=at_ps,
                    func=mybir.ActivationFunctionType.Exp,
                    bias=nmax, scale=1.0,
                    accum_out=rsum,
                )
                rinv = small_pool.tile([D, 1], F32, tag="rinv")
                nc.vector.reciprocal(out=rinv, in_=rsum)
                sc_sb = small_pool.tile([D, D], F32, tag="sc_sb")
                nc.vector.tensor_scalar_mul(out=sc_sb, in0=e_sb, scalar1=rinv)

                tp_tiles[2] = make_tp(natP2, 2)
                tp_tiles[3] = make_tp(natP2, 3)

                # ---- M^T replicated on 4 partition blocks ----
                m4_ps = acc_psum.tile([128, D], F32, tag="acc")
                nc.tensor.matmul(m4_ps, lhsT=wvt_rep, rhs=sc_sb)
                m4_sb = small_pool.tile([128, D], F32, tag="m4_sb")
                nc.scalar.copy(out=m4_sb, in_=m4_ps)
                # Block-diagonal lhsT for the output matmuls, with columns
                # ordered (q, g) so the matmul output partition is 4q + c
                # (affine in the DRAM row of out[b]): one full-width matmul
                # per transpose batch. The mask multiply also performs the
                # f32r rounding.
                bd = small_pool.tile([128, 128], F32R, tag="bd")
                m4_bcast = bass.AP(
                    tensor=m4_sb.tensor,
                    offset=m4_sb.offset,
                    ap=[list(m4_sb.ap[0]), list(m4_sb.ap[1]), [0, 4]],
                )
                nc.gpsimd.tensor_mul(
                    out=bd.rearrange("p (q g) -> p q g", g=4),
                    in0=m4_bcast,
                    in1=qgmask.rearrange("p (q g) -> p q g", g=4),
                )

                # ---- output phase: trhs copies, block-diag matmuls, and
                # the PSUM->SBUF copies that scatter p -> 16p + l into the
                # contiguous store tile ----
                o_sb = osb_pool.tile([128, N_P, N_L], F32, tag="o_sb",
                                     name=f"osb_{b}")
                for t in range(4):
                    trhs = trhs_pool.tile([128, 512], F32R, tag="trhs")
                    tp_src = tp_tiles[t].bitcast(F32R)
                    if t % 2 == 0:
                        nc.scalar.copy(out=trhs, in_=tp_src)
                    else:
                        nc.vector.tensor_copy(out=trhs, in_=tp_src)
                    o_ps = o_psum.tile([128, 512], F32, tag="o")
                    nc.tensor.matmul(o_ps, lhsT=bd, rhs=trhs)
                    # o_ps[z, 128i + p] -> o_sb[z, p, 4t + i]
                    dst = o_sb[:, :, 4 * t:4 * (t + 1)]
                    src = o_ps.rearrange("z (i p) -> z p i", i=4)
                    if t % 2 == 0:
                        nc.vector.tensor_copy(out=dst, in_=src)
                    else:
                        nc.scalar.copy(out=dst, in_=src)
                o_sb_tiles[b] = o_sb

                # ---- store: one fully contiguous 1 MB DMA per example ----
                nc.gpsimd.dma_start(
                    out=out_t[b].rearrange("q (c f) -> (q c) f", c=N_C),
                    in_=o_sb.rearrange("z p l -> z (p l)"),
                )

    nc.compile()
    return nc


_CACHED_NC = None


def _get_nc():
    global _CACHED_NC
    if _CACHED_NC is None:
        _CACHED_NC = build_nc()
    return _CACHED_NC


def make_cst(wq, wk, wv):
    """[128, 352]: identity | tile(Wv,(4,1)) | Wq | Wk | (q,g) group mask."""
    cst = np.zeros((128, 352), dtype=np.float32)
    cst[:, 0:128] = np.eye(128, dtype=np.float32)
    cst[:, 128:160] = np.tile(wv, (4, 1))
    cst[0:D, 160:192] = wq
    cst[0:D, 192:224] = wk
    pblk = np.arange(128) // 32
    g = np.arange(128) % 4
    cst[:, 224:352] = (pblk[:, None] == g[None, :]).astype(np.float32)
    return cst


def kernel(x, Wq, Wk, Wv):
    x = np.ascontiguousarray(np.asarray(x, dtype=np.float32))
    wq = np.asarray(Wq, dtype=np.float32).reshape(D, D)
    wk = np.asarray(Wk, dtype=np.float32).reshape(D, D)
    wv = np.asarray(Wv, dtype=np.float32).reshape(D, D)
    assert x.shape == (B, S, D)
    cst = make_cst(wq, wk, wv)

    nc = _get_nc()
    eye = np.eye(128, dtype=np.float32)
    in_maps = [
        {
            "x": x[c * PER_CORE:(c + 1) * PER_CORE],
            "cst": cst,
            "eye": eye,
        }
        for c in range(N_CORES)
    ]
    res = run_bass_kernel_spmd(nc, in_maps, list(range(N_CORES)))
    out = np.concatenate([res.results[c]["out"] for c in range(N_CORES)], axis=0)
    return out


# revision 13
# speedup vs baseline: 1.4480x; 1.3863x over previous
"""Trainium2 Bass kernel for nn_Attention_49185965473844.

Math (per example b):
    q = x @ Wq ; k = x @ Wk ; v = x @ Wv          (x: [S, D], W*: [D, D], D=32)
    A[q,k]   = sum_s q[s,q] k[s,k]  = (Wq^T G Wk)[q,k],   G = x^T x   ([32, 32])
    scores   = softmax(A, axis=q)                 (normalize down columns)
    out[q,s] = sum_k scores[q,k] v[s,k] = (M @ x^T)[q,s], M = scores @ Wv^T

So the whole problem reduces to: one Gram matrix G = x^T x per example (the
only big contraction), a tiny 32x32 chain + softmax, and one [32,32] @ [32,S]
matmul against x^T (PE transposes of the resident x tile).

The kernel is HBM/DMA-bound (16 MB of unavoidable traffic per core), so the
SBUF layout of x is chosen to give large contiguous DMA descriptors on BOTH
the load and the store:

    s = 2048*c + 16*p + l,   c in [0,4), p in [0,128) (partition), l in [0,16)

  * load:  nat[p, (c,l,d)] = x[s,d]  -> per partition 4 runs of 16 rows
    = 2 KB contiguous each (vs 128 B in a plain "(c p) d" layout).
  * an ACT/DVE copy reorders+casts to natP[p, (l,c,d)] in bf16 (the PE
    moving/stationary operands need single-stride APs; bf16 keeps every PE
    op at 1 cyc/row and halves copy cost; measured end-to-end rel err ~2e-3
    vs the 2e-2 gate).
  * PE transpose of the [128, (c d)] block at fixed l gives
    T_l[(c,d), p] = x^T with partition group c = the TOP 2 bits of s.
  * block-diag matmul (bd columns ordered (q, g)) -> o[(q,c), p].
  * the mandatory PSUM->SBUF copy scatters columns p -> 16*p + l, so the
    assembled O_sb[(q,c), f] = out[q, 2048*c + f] stores as ONE fully
    contiguous 1 MB DMA per example.

The per-example work is software-pipelined so the PE never idles (HAM stays
at 2.4 GHz): iteration b runs gram+transposes of example b, the 32x32
chain/softmax of example b-1, and the output matmuls/stores of example b-2.

Sharding: pure data parallel over batch B=64 -> 8 examples per NeuronCore.
"""

import numpy as np
import ml_dtypes

import concourse.bass as bass
import concourse.bacc as bacc
import concourse.tile as tile
from concourse import mybir
from concourse.bass_utils import run_bass_kernel_spmd

N_CORES = 8
B, S, D = 64, 8192, 32
PER_CORE = B // N_CORES  # 8

F32 = mybir.dt.float32
F32R = mybir.dt.float32r
BF16 = mybir.dt.bfloat16

# numpy dtype of the "eye" input fed to the kernel (used by test harnesses)
_EYE_NP_DTYPE = ml_dtypes.bfloat16

N_C = 4    # s bits 11..12: partition group of the transposed tiles
N_L = 16   # s bits 0..3:  within-partition interleave (load run = 16 rows)
N_P = 128  # s bits 4..10: SBUF partition of the natural tile


def build_nc(n_ex=PER_CORE, seq=S):
    """Build the per-core Bass program. Same program runs on all 8 cores."""
    assert seq == N_C * N_P * N_L
    nc = bacc.Bacc("TRN2", target_bir_lowering=False, debug=False)
    x_t = nc.declare_dram_parameter("x", [n_ex, seq, D], F32R, isOutput=False)
    eye_t = nc.declare_dram_parameter("eye", [128, 128], BF16, isOutput=False)
    cst_t = nc.declare_dram_parameter("cst", [128, 352], F32, isOutput=False)
    out_t = nc.declare_dram_parameter("out", [n_ex, D, seq], F32, isOutput=True)

    with tile.TileContext(nc) as tc:
        with (
            tc.tile_pool(name="consts", bufs=1) as consts,
            tc.tile_pool(name="nat_pool", bufs=n_ex) as nat_pool,
            tc.tile_pool(name="natp_pool", bufs=3) as natp_pool,
            tc.tile_pool(name="trhs_pool", bufs=3) as trhs_pool,
            tc.tile_pool(name="osb_pool", bufs=4) as osb_pool,
            tc.tile_pool(name="small_pool", bufs=3) as small_pool,
            tc.tile_pool(name="gram_psum", bufs=2, space="PSUM") as gram_psum,
            tc.tile_pool(name="acc_psum", bufs=2, space="PSUM") as acc_psum,
            tc.tile_pool(name="tp_psum", bufs=2, space="PSUM") as tp_psum,
            tc.tile_pool(name="o_psum", bufs=2, space="PSUM") as o_psum,
        ):
            # ---- constants ----
            cst_sb = consts.tile([128, 352], F32)
            nc.sync.dma_start(out=cst_sb, in_=cst_t[:, :])
            identity = cst_sb[:, 0:128]
            wv4 = cst_sb[:, 128:160]       # np.tile(Wv, (4, 1))
            wq_sb = cst_sb[0:D, 160:192]
            wk_sb = cst_sb[0:D, 192:224]
            # qgmask[p, 4*q + g] = 1.0 iff p//32 == g
            qgmask = cst_sb[:, 224:352]
            # Wv replicated on 4 partition blocks, PE-transposed so that
            # wvt_rep[k, 32*j + d] = Wv[d, k].
            wvt_ps = acc_psum.tile([D, 128], F32, tag="acc")
            nc.tensor.transpose(wvt_ps, wv4, identity)
            wvt_rep = consts.tile([D, 128], F32)
            nc.scalar.copy(out=wvt_rep, in_=wvt_ps)
            ident_b = consts.tile([128, 128], BF16)
            nc.sync.dma_start(out=ident_b, in_=eye_t[:, :])

            def load_nat(b):
                # nat[p, c, l, d] = x[b, 2048c + 16p + l, d]; per partition
                # the (l, d) block is 16 rows = 2 KB contiguous in DRAM.
                nat = nat_pool.tile([128, N_C, N_L, D], F32R, tag="nat",
                                    name=f"nat_{b}")
                nc.sync.dma_start(
                    out=nat,
                    in_=x_t[b].rearrange("(c p l) d -> p c l d",
                                         c=N_C, p=N_P, l=N_L),
                )
                return nat

            # All example loads are queued upfront (x is SBUF-resident for
            # the whole kernel); stores ride the gpsimd SWDGE queue so load
            # and store packets interleave at the DMA engines.
            nats = [load_nat(b) for b in range(n_ex)]

            def make_reorder(b):
                """(c,l,d) -> (l,c,d) + fp32->bf16 cast, split ACT/DVE."""
                natP = natp_pool.tile([128, N_L, N_C, D], BF16, tag="natp",
                                      name=f"natp_{b}")
                src = nats[b].rearrange("p c l d -> p l c d")
                h = N_L // 2
                nc.vector.tensor_copy(out=natP[:, 0:h], in_=src[:, 0:h])
                nc.scalar.copy(out=natP[:, h:N_L], in_=src[:, h:N_L])
                return natP

            # per-example state carried across pipeline stages
            st = [dict() for _ in range(n_ex)]
            natPs = {0: make_reorder(0)}

            def make_tp(b, t):
                """T[(c,d), p] = x[2048c + 16p + (4t+i), d], i in 0..4."""
                natP2 = st[b]["natP2"]
                tp_ps = tp_psum.tile([128, 512], BF16, tag="tp",
                                     name=f"tp_{b}_{t}")
                for i in range(4):
                    l0 = 4 * t + i
                    nc.tensor.transpose(
                        tp_ps[:, 128 * i:128 * (i + 1)],
                        natP2[:, 128 * l0:128 * (l0 + 1)],
                        ident_b,
                    )
                st[b][f"tp{t}"] = tp_ps

            def copy_trhs(b, t):
                """PSUM->SBUF copy of transpose batch t into the [128,2048]
                bf16 rhs tile for the output matmuls (iteration b+2)."""
                if t == 0:
                    st[b]["trhs"] = trhs_pool.tile(
                        [128, 2048], BF16, tag="trhs", name=f"trhs_{b}"
                    )
                dst = st[b]["trhs"][:, 512 * t:512 * (t + 1)]
                if t % 2 == 0:
                    nc.scalar.copy(out=dst, in_=st[b][f"tp{t}"])
                else:
                    nc.vector.tensor_copy(out=dst, in_=st[b][f"tp{t}"])
                st[b].pop(f"tp{t}")

            for it in range(n_ex + 2):
                b = it            # gram/transpose stage
                b1 = it - 1       # chain/softmax stage
                b2 = it - 2       # output-matmul/store stage

                # ---- PE: gram(b): 16 accumulating [128,128] bf16 self
                # products; diagonal 32x32 blocks sum to G = x^T x ----
                if b < n_ex:
                    natP = natPs.pop(b)
                    natP2 = natP.rearrange("p l c d -> p (l c d)")
                    st[b]["natP2"] = natP2
                    # f32r (TF32) gram from the fp32-loaded tile: the bf16
                    # x is too coarse for the softmax scores (2.3e-2 rel
                    # err); f32r restores it to ~2e-3. f32r needs moving
                    # dim >= 256 for 1 cyc/row, so quads stream a 256-wide
                    # block (the right half is a discarded cross product).
                    nat2 = nats[b].rearrange("p c l d -> p (c l d)")
                    gram_ps = gram_psum.tile([128, 256], F32, tag="gram",
                                             name=f"gram_{b}")
                    n_blk = (N_C * N_L * D) // 128  # 16
                    for t in range(n_blk):
                        last = t == n_blk - 1
                        width = 128 if last else 256
                        nc.tensor.matmul(
                            gram_ps[:, 0:width],
                            lhsT=nat2[:, 128 * t:128 * (t + 1)],
                            rhs=nat2[:, 128 * t:128 * t + width],
                            start=(t == 0),
                            stop=last,
                            skip_group_check=True,
                        )
                    st[b]["gram_ps"] = gram_ps

                # ---- PE: fold(b1) + t2(b1) (chain of example b-1) ----
                if 0 <= b1 < n_ex:
                    s1 = st[b1]
                    g_ps = acc_psum.tile([D, D], F32, tag="acc")
                    for j in range(4):
                        nc.tensor.matmul(
                            g_ps,
                            lhsT=identity[:, 32 * j:32 * (j + 1)],
                            rhs=s1["gram_sb"][:, 32 * j:32 * (j + 1)],
                            start=(j == 0),
                            stop=(j == 3),
                        )
                    g_sb = small_pool.tile([D, D], F32, tag="g_sb")
                    nc.scalar.copy(out=g_sb, in_=g_ps)
                    # A^T = Wk^T (G Wq);  G symmetric so lhsT=G works
                    t2_ps = acc_psum.tile([D, D], F32, tag="acc")
                    nc.tensor.matmul(t2_ps, lhsT=g_sb, rhs=wq_sb)
                    t2_sb = small_pool.tile([D, D], F32, tag="t2_sb")
                    nc.scalar.copy(out=t2_sb, in_=t2_ps)
                    s1["t2_sb"] = t2_sb

                if b < n_ex:
                    make_tp(b, 0)

                if 0 <= b1 < n_ex:
                    s1 = st[b1]
                    at_ps = acc_psum.tile([D, D], F32, tag="acc")
                    nc.tensor.matmul(at_ps, lhsT=wk_sb, rhs=s1["t2_sb"])

                if b < n_ex:
                    copy_trhs(b, 0)
                    # gram fold copy: frees the gram PSUM bank for b+1 and
                    # feeds iteration b+1's fold matmuls
                    gram_sb = small_pool.tile([128, 128], F32, tag="gram_sb")
                    nc.scalar.copy(out=gram_sb, in_=st[b]["gram_ps"][:, 0:128])
                    st[b]["gram_sb"] = gram_sb

                if 0 <= b1 < n_ex:
                    # softmax over q (free dim of A^T) on DVE/ACT
                    s1 = st[b1]
                    nmax = small_pool.tile([D, 1], F32, tag="nmax")
                    nc.vector.reduce_max(
                        out=nmax, in_=at_ps, axis=mybir.AxisListType.X,
                        negate=True,
                    )
                    e_sb = small_pool.tile([D, D], F32, tag="e_sb")
                    rsum = small_pool.tile([D, 1], F32, tag="rsum")
                    nc.scalar.activation(
                        out=e_sb, in_=at_ps,
                        func=mybir.ActivationFunctionType.Exp,
                        bias=nmax, scale=1.0,
                        accum_out=rsum,
                    )
                    rinv = small_pool.tile([D, 1], F32, tag="rinv")
                    nc.vector.reciprocal(out=rinv, in_=rsum)
                    sc_sb = small_pool.tile([D, D], F32, tag="sc_sb")
                    nc.vector.tensor_scalar_mul(out=sc_sb, in0=e_sb,
                                                scalar1=rinv)
                    s1["sc_sb"] = sc_sb

                if b < n_ex:
                    make_tp(b, 1)
                    make_tp(b, 2)
                    copy_trhs(b, 1)

                # ---- PE: M^T(b1) + bd mask-mul on gpsimd ----
                if 0 <= b1 < n_ex:
                    s1 = st[b1]
                    m4_ps = acc_psum.tile([128, D], F32, tag="acc")
                    nc.tensor.matmul(m4_ps, lhsT=wvt_rep, rhs=s1["sc_sb"])
                    m4_sb = small_pool.tile([128, D], F32, tag="m4_sb")
                    nc.scalar.copy(out=m4_sb, in_=m4_ps)
                    # Block-diagonal lhsT for the output matmuls, columns
                    # ordered (q, g) so the output partition 4q + c is affine
                    # in the DRAM row of out[b]. The mask multiply also casts
                    # to bf16.
                    bd = small_pool.tile([128, 128], BF16, tag="bd")
                    m4_bcast = bass.AP(
                        tensor=m4_sb.tensor,
                        offset=m4_sb.offset,
                        ap=[list(m4_sb.ap[0]), list(m4_sb.ap[1]), [0, 4]],
                    )
                    nc.gpsimd.tensor_mul(
                        out=bd.rearrange("p (q g) -> p q g", g=4),
                        in0=m4_bcast,
                        in1=qgmask.rearrange("p (q g) -> p q g", g=4),
                    )
                    s1["bd"] = bd

                # prefetch: reorder+cast of example b+1 on DVE/ACT (placed
                # after the softmax/chain copies so it never delays them)
                if 0 <= b + 1 < n_ex:
                    natPs[b + 1] = make_reorder(b + 1)

                if b < n_ex:
                    make_tp(b, 3)
                    copy_trhs(b, 2)
                    copy_trhs(b, 3)

                # ---- PE: output matmuls of example b-2 ----
                if 0 <= b2 < n_ex:
                    s2 = st[b2]
                    o_sb = osb_pool.tile([128, N_P, N_L], F32, tag="o_sb",
                                         name=f"osb_{b2}")
                    for t in range(4):
                        o_ps = o_psum.tile([128, 512], F32, tag="o")
                        nc.tensor.matmul(
                            o_ps, lhsT=s2["bd"],
                            rhs=s2["trhs"][:, 512 * t:512 * (t + 1)],
                        )
                        # o_ps[z, 128i + p] -> o_sb[z, p, 4t + i]
                        dst = o_sb[:, :, 4 * t:4 * (t + 1)]
                        src = o_ps.rearrange("z (i p) -> z p i", i=4)
                        if t % 2 == 0:
                            nc.vector.tensor_copy(out=dst, in_=src)
                        else:
                            nc.scalar.copy(out=dst, in_=src)
                    # store: one fully contiguous 1 MB DMA per example
                    nc.gpsimd.dma_start(
                        out=out_t[b2].rearrange("q (c f) -> (q c) f", c=N_C),
                        in_=o_sb.rearrange("z p l -> z (p l)"),
                    )

    nc.compile()
    return nc


_CACHED_NC = None


def _get_nc():
    global _CACHED_NC
    if _CACHED_NC is None:
        _CACHED_NC = build_nc()
    return _CACHED_NC


def make_cst(wq, wk, wv):
    """[128, 352]: identity | tile(Wv,(4,1)) | Wq | Wk | (q,g) group mask."""
    cst = np.zeros((128, 352), dtype=np.float32)
    cst[:, 0:128] = np.eye(128, dtype=np.float32)
    cst[:, 128:160] = np.tile(wv, (4, 1))
    cst[0:D, 160:192] = wq
    cst[0:D, 192:224] = wk
    pblk = np.arange(128) // 32
    g = np.arange(128) % 4
    cst[:, 224:352] = (pblk[:, None] == g[None, :]).astype(np.float32)
    return cst


def kernel(x, Wq, Wk, Wv):
    x = np.ascontiguousarray(np.asarray(x, dtype=np.float32))
    wq = np.asarray(Wq, dtype=np.float32).reshape(D, D)
    wk = np.asarray(Wk, dtype=np.float32).reshape(D, D)
    wv = np.asarray(Wv, dtype=np.float32).reshape(D, D)
    assert x.shape == (B, S, D)
    cst = make_cst(wq, wk, wv)

    nc = _get_nc()
    eye = np.eye(128, dtype=ml_dtypes.bfloat16)
    in_maps = [
        {
            "x": x[c * PER_CORE:(c + 1) * PER_CORE],
            "cst": cst,
            "eye": eye,
        }
        for c in range(N_CORES)
    ]
    res = run_bass_kernel_spmd(nc, in_maps, list(range(N_CORES)))
    out = np.concatenate([res.results[c]["out"] for c in range(N_CORES)], axis=0)
    return out


# revision 14
# speedup vs baseline: 1.8052x; 1.2467x over previous
"""Trainium2 Bass kernel for nn_Attention_49185965473844.

Math (per example b):
    q = x @ Wq ; k = x @ Wk ; v = x @ Wv          (x: [S, D], W*: [D, D], D=32)
    A[q,k]   = sum_s q[s,q] k[s,k]  = (Wq^T G Wk)[q,k],   G = x^T x   ([32, 32])
    scores   = softmax(A, axis=q)                 (normalize down columns)
    out[q,s] = sum_k scores[q,k] v[s,k] = (M @ x^T)[q,s], M = scores @ Wv^T

So the whole problem reduces to: one Gram matrix G = x^T x per example (the
only big contraction), a tiny 32x32 chain + softmax, and one [32,32] @ [32,S]
matmul against x^T (PE transposes of the resident x tile).

The kernel is HBM/DMA-bound (16 MB of unavoidable traffic per core), so the
SBUF layout of x is chosen to give large contiguous DMA descriptors on BOTH
the load and the store:

    s = 2048*c + 16*p + l,   c in [0,4), p in [0,128) (partition), l in [0,16)

  * load:  nat[p, (c,l,d)] = x[s,d]  -> per partition 4 runs of 16 rows
    = 2 KB contiguous each (vs 128 B in a plain "(c p) d" layout).
  * an ACT/DVE copy reorders+casts to natP[p, (l,c,d)] in bf16 (the PE
    moving/stationary operands need single-stride APs; bf16 keeps every PE
    op at 1 cyc/row and halves copy cost; measured end-to-end rel err ~2e-3
    vs the 2e-2 gate).
  * PE transpose of the [128, (c d)] block at fixed l gives
    T_l[(c,d), p] = x^T with partition group c = the TOP 2 bits of s.
  * block-diag matmul (bd columns ordered (q, g)) -> o[(q,c), p].
  * the mandatory PSUM->SBUF copy scatters columns p -> 16*p + l, so the
    assembled O_sb[(q,c), f] = out[q, 2048*c + f] stores as ONE fully
    contiguous 1 MB DMA per example.

The per-example work is software-pipelined so the PE never idles (HAM stays
at 2.4 GHz): iteration b runs gram+transposes of example b, the 32x32
chain/softmax of example b-1, and the output matmuls/stores of example b-2.

Sharding: pure data parallel over batch B=64 -> 8 examples per NeuronCore.
"""

import numpy as np
import ml_dtypes

import concourse.bass as bass
import concourse.bacc as bacc
import concourse.tile as tile
from concourse import mybir
from concourse.bass_utils import run_bass_kernel_spmd

N_CORES = 8
B, S, D = 64, 8192, 32
PER_CORE = B // N_CORES  # 8

F32 = mybir.dt.float32
FP16 = mybir.dt.float16

# numpy dtype of the "eye" input fed to the kernel (used by test harnesses)
_EYE_NP_DTYPE = np.float16

N_C = 4    # s bits 11..12: partition group of the transposed tiles
N_L = 16   # s bits 0..3:  within-partition interleave (load run = 16 rows)
N_P = 128  # s bits 4..10: SBUF partition of the natural tile


def build_nc(n_ex=PER_CORE, seq=S):
    """Build the per-core Bass program. Same program runs on all 8 cores."""
    assert seq == N_C * N_P * N_L
    nc = bacc.Bacc("TRN2", target_bir_lowering=False, debug=False)
    x_t = nc.declare_dram_parameter("x", [n_ex, seq, D], F32, isOutput=False)
    eye_t = nc.declare_dram_parameter("eye", [128, 128], FP16, isOutput=False)
    cst_t = nc.declare_dram_parameter("cst", [128, 352], F32, isOutput=False)
    out_t = nc.declare_dram_parameter("out", [n_ex, D, seq], F32, isOutput=True)

    with tile.TileContext(nc) as tc:
        with (
            tc.tile_pool(name="consts", bufs=1) as consts,
            tc.tile_pool(name="nat_pool", bufs=n_ex) as nat_pool,
            tc.tile_pool(name="natp_pool", bufs=3) as natp_pool,
            tc.tile_pool(name="trhs_pool", bufs=3) as trhs_pool,
            tc.tile_pool(name="osb_pool", bufs=4) as osb_pool,
            tc.tile_pool(name="small_pool", bufs=3) as small_pool,
            tc.tile_pool(name="gram_psum", bufs=2, space="PSUM") as gram_psum,
            tc.tile_pool(name="acc_psum", bufs=2, space="PSUM") as acc_psum,
            tc.tile_pool(name="tp_psum", bufs=2, space="PSUM") as tp_psum,
            tc.tile_pool(name="o_psum", bufs=2, space="PSUM") as o_psum,
        ):
            # ---- constants ----
            cst_sb = consts.tile([128, 352], F32)
            nc.sync.dma_start(out=cst_sb, in_=cst_t[:, :])
            identity = cst_sb[:, 0:128]
            wv4 = cst_sb[:, 128:160]       # np.tile(Wv, (4, 1))
            wq_sb = cst_sb[0:D, 160:192]
            wk_sb = cst_sb[0:D, 192:224]
            # qgmask[p, 4*q + g] = 1.0 iff p//32 == g
            qgmask = cst_sb[:, 224:352]
            # Wv replicated on 4 partition blocks, PE-transposed so that
            # wvt_rep[k, 32*j + d] = Wv[d, k].
            wvt_ps = acc_psum.tile([D, 128], F32, tag="acc")
            nc.tensor.transpose(wvt_ps, wv4, identity)
            wvt_rep = consts.tile([D, 128], F32)
            nc.scalar.copy(out=wvt_rep, in_=wvt_ps)
            ident_b = consts.tile([128, 128], FP16)
            nc.sync.dma_start(out=ident_b, in_=eye_t[:, :])

            def load_nat(b):
                # nat[p, c, l, d] = x[b, 2048c + 16p + l, d]; per partition
                # the (l, d) block is 16 rows = 2 KB contiguous in DRAM.
                nat = nat_pool.tile([128, N_C, N_L, D], F32, tag="nat",
                                    name=f"nat_{b}")
                nc.sync.dma_start(
                    out=nat,
                    in_=x_t[b].rearrange("(c p l) d -> p c l d",
                                         c=N_C, p=N_P, l=N_L),
                )
                return nat

            # All example loads are queued upfront (x is SBUF-resident for
            # the whole kernel); stores ride the gpsimd SWDGE queue so load
            # and store packets interleave at the DMA engines.
            nats = [load_nat(b) for b in range(n_ex)]

            def make_reorder(b):
                """(c,l,d) -> (l,c,d) + fp32->bf16 cast, split ACT/DVE."""
                natP = natp_pool.tile([128, N_L, N_C, D], FP16, tag="natp",
                                      name=f"natp_{b}")
                src = nats[b].rearrange("p c l d -> p l c d")
                h = N_L // 2
                nc.vector.tensor_copy(out=natP[:, 0:h], in_=src[:, 0:h])
                nc.scalar.copy(out=natP[:, h:N_L], in_=src[:, h:N_L])
                return natP

            # per-example state carried across pipeline stages
            st = [dict() for _ in range(n_ex)]
            natPs = {0: make_reorder(0)}

            def make_tp(b, t):
                """T[(c,d), p] = x[2048c + 16p + (4t+i), d], i in 0..4."""
                natP2 = st[b]["natP2"]
                tp_ps = tp_psum.tile([128, 512], FP16, tag="tp",
                                     name=f"tp_{b}_{t}")
                for i in range(4):
                    l0 = 4 * t + i
                    nc.tensor.transpose(
                        tp_ps[:, 128 * i:128 * (i + 1)],
                        natP2[:, 128 * l0:128 * (l0 + 1)],
                        ident_b,
                    )
                st[b][f"tp{t}"] = tp_ps

            def copy_trhs(b, t):
                """PSUM->SBUF copy of transpose batch t into the [128,2048]
                bf16 rhs tile for the output matmuls (iteration b+2)."""
                if t == 0:
                    st[b]["trhs"] = trhs_pool.tile(
                        [128, 2048], FP16, tag="trhs", name=f"trhs_{b}"
                    )
                dst = st[b]["trhs"][:, 512 * t:512 * (t + 1)]
                if t % 2 == 0:
                    nc.scalar.copy(out=dst, in_=st[b][f"tp{t}"])
                else:
                    nc.vector.tensor_copy(out=dst, in_=st[b][f"tp{t}"])
                st[b].pop(f"tp{t}")

            for it in range(n_ex + 2):
                b = it            # gram/transpose stage
                b1 = it - 1       # chain/softmax stage
                b2 = it - 2       # output-matmul/store stage

                # ---- PE: gram(b): 16 accumulating [128,128] bf16 self
                # products; diagonal 32x32 blocks sum to G = x^T x ----
                if b < n_ex:
                    natP = natPs.pop(b)
                    natP2 = natP.rearrange("p l c d -> p (l c d)")
                    st[b]["natP2"] = natP2
                    # fp16 gram (10-bit mantissa ~ f32r quality, rel err
                    # 8e-4 end to end; bf16's 8 bits gave 2.3e-2): 16
                    # accumulating [128,128] self products, each 1 cyc/row
                    # with FWL weight loads. Diagonal 32x32 blocks sum to G.
                    gram_ps = gram_psum.tile([128, 128], F32, tag="gram",
                                             name=f"gram_{b}")
                    n_blk = (N_C * N_L * D) // 128  # 16
                    for t in range(n_blk):
                        nc.tensor.matmul(
                            gram_ps,
                            lhsT=natP2[:, 128 * t:128 * (t + 1)],
                            rhs=natP2[:, 128 * t:128 * (t + 1)],
                            start=(t == 0),
                            stop=(t == n_blk - 1),
                        )
                    st[b]["gram_ps"] = gram_ps

                # ---- PE: fold(b1) + t2(b1) (chain of example b-1) ----
                if 0 <= b1 < n_ex:
                    s1 = st[b1]
                    g_ps = acc_psum.tile([D, D], F32, tag="acc")
                    for j in range(4):
                        nc.tensor.matmul(
                            g_ps,
                            lhsT=identity[:, 32 * j:32 * (j + 1)],
                            rhs=s1["gram_sb"][:, 32 * j:32 * (j + 1)],
                            start=(j == 0),
                            stop=(j == 3),
                        )
                    g_sb = small_pool.tile([D, D], F32, tag="g_sb")
                    nc.scalar.copy(out=g_sb, in_=g_ps)
                    # A^T = Wk^T (G Wq);  G symmetric so lhsT=G works
                    t2_ps = acc_psum.tile([D, D], F32, tag="acc")
                    nc.tensor.matmul(t2_ps, lhsT=g_sb, rhs=wq_sb)
                    t2_sb = small_pool.tile([D, D], F32, tag="t2_sb")
                    nc.scalar.copy(out=t2_sb, in_=t2_ps)
                    s1["t2_sb"] = t2_sb

                if b < n_ex:
                    make_tp(b, 0)

                if 0 <= b1 < n_ex:
                    s1 = st[b1]
                    at_ps = acc_psum.tile([D, D], F32, tag="acc")
                    nc.tensor.matmul(at_ps, lhsT=wk_sb, rhs=s1["t2_sb"])

                if b < n_ex:
                    copy_trhs(b, 0)
                    # gram fold copy: frees the gram PSUM bank for b+1 and
                    # feeds iteration b+1's fold matmuls
                    gram_sb = small_pool.tile([128, 128], F32, tag="gram_sb")
                    nc.scalar.copy(out=gram_sb, in_=st[b]["gram_ps"])
                    st[b]["gram_sb"] = gram_sb

                if 0 <= b1 < n_ex:
                    # softmax over q (free dim of A^T) on DVE/ACT
                    s1 = st[b1]
                    nmax = small_pool.tile([D, 1], F32, tag="nmax")
                    nc.vector.reduce_max(
                        out=nmax, in_=at_ps, axis=mybir.AxisListType.X,
                        negate=True,
                    )
                    e_sb = small_pool.tile([D, D], F32, tag="e_sb")
                    rsum = small_pool.tile([D, 1], F32, tag="rsum")
                    nc.scalar.activation(
                        out=e_sb, in_=at_ps,
                        func=mybir.ActivationFunctionType.Exp,
                        bias=nmax, scale=1.0,
                        accum_out=rsum,
                    )
                    rinv = small_pool.tile([D, 1], F32, tag="rinv")
                    nc.vector.reciprocal(out=rinv, in_=rsum)
                    sc_sb = small_pool.tile([D, D], F32, tag="sc_sb")
                    nc.vector.tensor_scalar_mul(out=sc_sb, in0=e_sb,
                                                scalar1=rinv)
                    s1["sc_sb"] = sc_sb

                if b < n_ex:
                    make_tp(b, 1)
                    make_tp(b, 2)
                    copy_trhs(b, 1)

                # ---- PE: M^T(b1) + bd mask-mul on gpsimd ----
                if 0 <= b1 < n_ex:
                    s1 = st[b1]
                    m4_ps = acc_psum.tile([128, D], F32, tag="acc")
                    nc.tensor.matmul(m4_ps, lhsT=wvt_rep, rhs=s1["sc_sb"])
                    m4_sb = small_pool.tile([128, D], F32, tag="m4_sb")
                    nc.scalar.copy(out=m4_sb, in_=m4_ps)
                    # Block-diagonal lhsT for the output matmuls, columns
                    # ordered (q, g) so the output partition 4q + c is affine
                    # in the DRAM row of out[b]. The mask multiply also casts
                    # to bf16.
                    bd = small_pool.tile([128, 128], FP16, tag="bd")
                    m4_bcast = bass.AP(
                        tensor=m4_sb.tensor,
                        offset=m4_sb.offset,
                        ap=[list(m4_sb.ap[0]), list(m4_sb.ap[1]), [0, 4]],
                    )
                    nc.gpsimd.tensor_mul(
                        out=bd.rearrange("p (q g) -> p q g", g=4),
                        in0=m4_bcast,
                        in1=qgmask.rearrange("p (q g) -> p q g", g=4),
                    )
                    s1["bd"] = bd

                # prefetch: reorder+cast of example b+1 on DVE/ACT (placed
                # after the softmax/chain copies so it never delays them)
                if 0 <= b + 1 < n_ex:
                    natPs[b + 1] = make_reorder(b + 1)

                if b < n_ex:
                    make_tp(b, 3)
                    copy_trhs(b, 2)
                    copy_trhs(b, 3)

                # ---- PE: output matmuls of example b-2 ----
                if 0 <= b2 < n_ex:
                    s2 = st[b2]
                    o_sb = osb_pool.tile([128, N_P, N_L], F32, tag="o_sb",
                                         name=f"osb_{b2}")
                    for t in range(4):
                        o_ps = o_psum.tile([128, 512], F32, tag="o")
                        nc.tensor.matmul(
                            o_ps, lhsT=s2["bd"],
                            rhs=s2["trhs"][:, 512 * t:512 * (t + 1)],
                        )
                        # o_ps[z, 128i + p] -> o_sb[z, p, 4t + i]
                        dst = o_sb[:, :, 4 * t:4 * (t + 1)]
                        src = o_ps.rearrange("z (i p) -> z p i", i=4)
                        if t % 2 == 0:
                            nc.vector.tensor_copy(out=dst, in_=src)
                        else:
                            nc.scalar.copy(out=dst, in_=src)
                    # store: one fully contiguous 1 MB DMA per example
                    nc.gpsimd.dma_start(
                        out=out_t[b2].rearrange("q (c f) -> (q c) f", c=N_C),
                        in_=o_sb.rearrange("z p l -> z (p l)"),
                    )

    nc.compile()
    return nc


_CACHED_NC = None


def _get_nc():
    global _CACHED_NC
    if _CACHED_NC is None:
        _CACHED_NC = build_nc()
    return _CACHED_NC


def make_cst(wq, wk, wv):
    """[128, 352]: identity | tile(Wv,(4,1)) | Wq | Wk | (q,g) group mask."""
    cst = np.zeros((128, 352), dtype=np.float32)
    cst[:, 0:128] = np.eye(128, dtype=np.float32)
    cst[:, 128:160] = np.tile(wv, (4, 1))
    cst[0:D, 160:192] = wq
    cst[0:D, 192:224] = wk
    pblk = np.arange(128) // 32
    g = np.arange(128) % 4
    cst[:, 224:352] = (pblk[:, None] == g[None, :]).astype(np.float32)
    return cst


def kernel(x, Wq, Wk, Wv):
    x = np.ascontiguousarray(np.asarray(x, dtype=np.float32))
    wq = np.asarray(Wq, dtype=np.float32).reshape(D, D)
    wk = np.asarray(Wk, dtype=np.float32).reshape(D, D)
    wv = np.asarray(Wv, dtype=np.float32).reshape(D, D)
    assert x.shape == (B, S, D)
    cst = make_cst(wq, wk, wv)

    nc = _get_nc()
    eye = np.eye(128, dtype=ml_dtypes.bfloat16)
    in_maps = [
        {
            "x": x[c * PER_CORE:(c + 1) * PER_CORE],
            "cst": cst,
            "eye": eye,
        }
        for c in range(N_CORES)
    ]
    res = run_bass_kernel_spmd(nc, in_maps, list(range(N_CORES)))
    out = np.concatenate([res.results[c]["out"] for c in range(N_CORES)], axis=0)
    return out
